# revision 23
# baseline (speedup 1.0000x reference)
"""Paged GQA attention (prefill + decode) for 8 Trainium2 NeuronCores.

Sharding: tensor-parallel over kv-heads. Core c owns kv-head c and its 4 GQA
query heads. Block tables / context lens are baked into the program (compiled
per call), so all control flow and gather addresses are static.

v2 design (vs the 219us baseline):
  - all prefill operands fp16: halves input DMA and enables FWL weight loads
    (fp32r stationaries can't use FWL, serializing 107ns LDWEIGHTS per matmul)
  - QK matmuls causally column-sliced (25% fewer PE columns)
  - exp merged across 2 query heads per instruction (each ACTIVATE carries a
    ~352-cycle fixed overhead; merging halves the count) with causal column
    skipping via 3D strided APs
  - outputs written unnormalized (with the ones-column row sums) as fp16;
    the division happens on host: frees ~90us of VectorE work and halves
    output DMA
  - decode uses fp8e4m3 KV packed at 128-token granularity, loaded in 2 large
    DMAs at program start (instead of 64 small ones), and runs as a separate
    phase at the end so prefill PSUM pools can be released and reused
"""

import sys

if "/opt/trn_rl_repo" not in sys.path:
    sys.path.insert(0, "/opt/trn_rl_repo")

import numpy as np
import ml_dtypes

import concourse.bass as bass  # noqa: F401  (registers AP machinery)
import concourse.mybir as mybir
import concourse.tile as tile
from concourse import bacc
from concourse.bass_utils import run_bass_kernel_spmd

NUM_HEADS = 32
NUM_KV_HEADS = 8
HEAD_DIM = 128
GQA = NUM_HEADS // NUM_KV_HEADS  # 4
SCALE = 0.08838834764831845
NUM_SEQS = 4
SEQLEN = 1024
N_PREFILL = NUM_SEQS * SEQLEN  # 4096
DECODE_BATCH = 32
NUM_BLOCKS = 256
BLOCK_SIZE = 256
MAX_BLOCKS = 8
TOTAL = N_PREFILL + DECODE_BATCH  # 4128
N_CORES = 8
MAX_KTILES = 16  # ceil(2047/128)

F32 = mybir.dt.float32
BF16 = mybir.dt.bfloat16
FP16 = mybir.dt.float16
FP8 = mybir.dt.float8e4
EXP = mybir.ActivationFunctionType.Exp

NP_FP8 = ml_dtypes.float8_e4m3fn

_program_cache: dict[bytes, object] = {}


def _build_program(ctx_lens: np.ndarray):
    """Build + finalize the (SPMD-identical) Bass program for one core."""
    nc = bacc.Bacc("TRN2", target_bir_lowering=False)

    # ---- static decode geometry (baked) ----
    ntiles_b = [-(-int(ctx_lens[b]) // 128) for b in range(DECODE_BATCH)]
    tile_off = [0]
    for nt in ntiles_b:
        tile_off.append(tile_off[-1] + nt)
    tot_tiles = tile_off[-1]

    qpreT = nc.dram_tensor("qpreT", [NUM_SEQS, 2, HEAD_DIM, 2, SEQLEN], FP16,
                           kind="ExternalInput")
    kpreT = nc.dram_tensor("kpreT", [NUM_SEQS, HEAD_DIM, SEQLEN], FP16,
                           kind="ExternalInput")
    vpre1 = nc.dram_tensor(
        "vpre1", [NUM_SEQS, 128, SEQLEN // 128, HEAD_DIM + 1], FP16,
        kind="ExternalInput")
    qdecT = nc.dram_tensor("qdecT", [HEAD_DIM, DECODE_BATCH * GQA], FP8,
                           kind="ExternalInput")
    kdec = nc.dram_tensor("kdec", [HEAD_DIM, tot_tiles * 128], FP8,
                          kind="ExternalInput")
    vdec = nc.dram_tensor("vdec", [128, tot_tiles, HEAD_DIM + 1], FP8,
                          kind="ExternalInput")
    trimask = nc.dram_tensor("trimask", [128, 128], FP16, kind="ExternalInput")
    tailmask = nc.dram_tensor("tailmask", [128, DECODE_BATCH], F32,
                              kind="ExternalInput")
    # unnormalized prefill out: [s, hp, c, 128 q, hh*4+ml, 129]
    preout = nc.dram_tensor(
        "preout", [NUM_SEQS, 2, 2, 128, 8, HEAD_DIM + 1], FP16,
        kind="ExternalOutput")
    # unnormalized decode out: [4 gqa, 32 seq, 129]
    ddec = nc.dram_tensor("ddec", [GQA, DECODE_BATCH, HEAD_DIM + 1], F32,
                          kind="ExternalOutput")

    with tile.TileContext(nc) as tc:
        with tc.tile_pool(name="consts", bufs=1) as consts, \
             tc.tile_pool(name="kv8", bufs=1) as kv8_pool:
            tri = consts.tile([128, 128], FP16)
            nc.sync.dma_start(tri, trimask[:, :])
            tail_s = consts.tile([128, DECODE_BATCH], F32)
            with tc.tile_wait_until(0.012):
                nc.sync.dma_start(tail_s, tailmask[:, :])
            qdec_s = consts.tile([HEAD_DIM, DECODE_BATCH * GQA], FP8)
            with tc.tile_wait_until(0.012):
                nc.sync.dma_start(qdec_s, qdecT[:, :])
            kp = kv8_pool.tile([HEAD_DIM, tot_tiles * 128], FP8, name="kp")
            vp = kv8_pool.tile([128, tot_tiles, HEAD_DIM + 1], FP8, name="vp")
            kv_prefetch_done = [False]
            # HAM warmup: ~5us of dummy back-to-back matmuls while the real
            # inputs stream in, so the PE clock gate is already at 8/8 when
            # the first real matmul issues (saves ~10us of half-clock start)
            wz = consts.tile([128, 512], FP16, name="wz")
            nc.vector.memset(wz, 0.0)

            # ---------------- prefill + interleaved decode ----------------
            # bufs cover the whole problem: all inputs prefetch at t=0 so no
            # mid-kernel load can be starved by the decode-KV stream
            with tc.tile_pool(name="kT", bufs=4) as kT_pool, \
                 tc.tile_pool(name="v1", bufs=4) as v1_pool, \
                 tc.tile_pool(name="qT", bufs=8) as qT_pool, \
                 tc.tile_pool(name="es", bufs=3) as e_pool, \
                 tc.tile_pool(name="stg", bufs=3) as stg_pool, \
                 tc.tile_pool(name="ed", bufs=2) as ed_pool, \
                 tc.tile_pool(name="dst", bufs=1) as dst_pool, \
                 tc.tile_pool(name="sc", bufs=2, space="PSUM") as s_pool, \
                 tc.tile_pool(name="ot", bufs=1, space="PSUM") as o_pool, \
                 tc.tile_pool(name="dec", bufs=1, space="PSUM") as dec_pool:
                dstage = dst_pool.tile([GQA, DECODE_BATCH, HEAD_DIM + 1], F32)

                def emit_chunk(s, hp, c, kT, v1, qT):
                    nj = 4 * (c + 1)
                    # 8 otile slots (hh*4+ml) packed 3/3/2 per PSUM bank
                    otA = o_pool.tile([128, 3, HEAD_DIM + 1], F32, name="otA",
                                      tag="otA")
                    otB = o_pool.tile([128, 3, HEAD_DIM + 1], F32, name="otB",
                                      tag="otB")
                    otC = o_pool.tile([128, 2, HEAD_DIM + 1], F32, name="otC",
                                      tag="otC")

                    def ot_slice(idx):
                        if idx < 3:
                            return otA[:, idx, :]
                        if idx < 6:
                            return otB[:, idx - 3, :]
                        return otC[:, idx - 6, :]

                    stage = stg_pool.tile([128, 8, HEAD_DIM + 1], FP16,
                                          name="stage")
                    es = []

                    def emit_av(j, e):
                        ml0 = j - 4 * c if j > 4 * c else 0
                        for hh in range(2):
                            for ml in range(ml0, 4):
                                idx = hh * 4 + ml
                                # start=True clears has_written for the WHOLE
                                # PSUM bank, so only the first group written to
                                # each bank (idx 0/3/6) may set it; the other
                                # groups' first writes land on cleared bits and
                                # overwrite anyway.
                                nc.tensor.matmul(
                                    ot_slice(idx),
                                    e[:, hh, ml * 128:(ml + 1) * 128],
                                    v1[:, j, :],
                                    start=(j == 0 and idx in (0, 3, 6)),
                                    stop=(j == 4 * c + ml),
                                    skip_group_check=True)
                                if j == 4 * c + ml:
                                    # evacuate as soon as this otile stops
                                    # (DVE: gpsimd can't read PSUM)
                                    nc.vector.tensor_copy(
                                        stage[:, idx, :], ot_slice(idx))

                    for j in range(nj):
                        off = 128 * (j - 4 * c) if j > 4 * c else 0
                        sc = s_pool.tile([128, 2, 512], F32, name="sc")
                        for hh in range(2):
                            nc.tensor.matmul(
                                sc[:, hh, off:],
                                kT[:, j * 128:(j + 1) * 128],
                                qT[:, hh, c * 512 + off:(c + 1) * 512],
                                start=True, stop=True)
                        e = e_pool.tile([128, 2, 512], FP16, name="e")
                        nc.scalar.activation(
                            e[:, :, off:], sc[:, :, off:], EXP, scale=SCALE)
                        if j >= 4 * c:
                            for hh in range(2):
                                nc.vector.tensor_mul(
                                    e[:, hh, off:off + 128],
                                    e[:, hh, off:off + 128],
                                    tri)
                        es.append((j, e))
                        if len(es) > 2:
                            emit_av(*es.pop(0))
                    while es:
                        emit_av(*es.pop(0))
                    nc.gpsimd.dma_start(preout[s, hp, c], stage)
                    if not kv_prefetch_done[0]:
                        # decode KV prefetch (9MB): delayed + split into
                        # per-4-seq slices. One huge descriptor starves the
                        # per-descriptor round-robin against the startup-
                        # critical q/k/v loads (first matmul then waits ~30us
                        # on its inputs); small descriptors share fairly and
                        # the wait hint keeps them off the critical window.
                        kv_prefetch_done[0] = True
                        for i, b0 in enumerate(range(0, DECODE_BATCH, 4)):
                            ta = tile_off[b0]
                            tb = tile_off[min(b0 + 4, DECODE_BATCH)]
                            with tc.tile_wait_until(0.030 + 0.006 * i):
                                nc.gpsimd.dma_start(
                                    kp[:, ta * 128:tb * 128],
                                    kdec[:, ta * 128:tb * 128])
                                nc.gpsimd.dma_start(
                                    vp[:, ta:tb, :], vdec[:, ta:tb, :])

                scw = dec_pool.tile([128, 512], F32, name="dec")
                for _ in range(12):
                    nc.tensor.matmul(scw, wz[:, 0:128], wz,
                                     start=True, stop=True,
                                     skip_group_check=True)

                NG = 4  # decode seqs per group
                SDW = NG * 4 * MAX_KTILES  # 256 score cols per group

                def emit_dec_qk(g):
                    # sd (cols 0:256) + od (cols 256:385) share one PSUM bank
                    # via flat offsets (PSUM slots pad to whole banks, so two
                    # tags won't pack)
                    dec = dec_pool.tile([128, 512], F32, name="dec")
                    nc.vector.memset(dec[:, 0:SDW], 0.0)
                    for i in range(NG):
                        b = g * NG + i
                        for t in range(ntiles_b[b]):
                            gt = tile_off[b] + t
                            o = i * 4 * MAX_KTILES + 4 * t
                            nc.tensor.matmul(
                                dec[:, o:o + 4],
                                kp[:, gt * 128:(gt + 1) * 128],
                                qdec_s[:, 4 * b:4 * b + 4],
                                start=True, stop=True,
                                skip_group_check=True)
                    ed = ed_pool.tile([128, SDW], FP16, name="ed")
                    nc.scalar.activation(ed, dec[:, 0:SDW], EXP, scale=SCALE)
                    return dec, ed

                def emit_dec_av(g, dec, ed):
                    od = dec[:, SDW:SDW + HEAD_DIM + 1]
                    for i in range(NG):
                        b = g * NG + i
                        nt = ntiles_b[b]
                        rem = int(ctx_lens[b]) - 128 * (nt - 1)
                        o = i * 4 * MAX_KTILES
                        if rem < 128:
                            nc.vector.tensor_scalar_mul(
                                ed[:, o + 4 * (nt - 1):o + 4 * nt],
                                ed[:, o + 4 * (nt - 1):o + 4 * nt],
                                tail_s[:, b:b + 1])
                        for t in range(nt):
                            gt = tile_off[b] + t
                            nc.tensor.matmul(
                                od[0:GQA, :],
                                ed[:, o + 4 * t:o + 4 * t + 4],
                                vp[:, gt, :],
                                start=(t == 0), stop=(t == nt - 1),
                                skip_group_check=True)
                        nc.vector.tensor_copy(dstage[:, b, :], od[0:GQA, :])

                # decode groups ride the back half of the prefill chunk list:
                # a full prefill chunk's matmuls sit between each group's QK
                # and its AV, hiding the exp latency, and the decode work
                # fills the PE slack of the ScalarE-paced prefill chunks.
                n_groups = DECODE_BATCH // NG
                chunk_no = [0]
                dec_pending = [None]

                def maybe_emit_decode():
                    k = chunk_no[0]
                    chunk_no[0] += 1
                    g = k - 5  # groups ride chunks 5..12: clear of both the
                    # startup-critical loads and the final-chunk tail
                    if dec_pending[0] is not None:
                        emit_dec_av(*dec_pending[0])
                        dec_pending[0] = None
                    if 0 <= g < n_groups:
                        dec, ed = emit_dec_qk(g)
                        dec_pending[0] = (g, dec, ed)

                for s in range(NUM_SEQS):
                    kT = kT_pool.tile([128, SEQLEN], FP16, name="kT")
                    nc.sync.dma_start(kT, kpreT[s])
                    for hp in range(2):
                        qT = qT_pool.tile([128, 2, SEQLEN], FP16, name="qT")
                        # split per head so the first matmul only waits on a
                        # 256KB load
                        nc.sync.dma_start(qT[:, 0, :], qpreT[s, hp, :, 0, :])
                        nc.sync.dma_start(qT[:, 1, :], qpreT[s, hp, :, 1, :])
                        if hp == 0:
                            v1 = v1_pool.tile(
                                [128, SEQLEN // 128, HEAD_DIM + 1], FP16,
                                name="v1")
                            nc.sync.dma_start(v1, vpre1[s])
                        for c in range(2):
                            emit_chunk(s, hp, c, kT, v1, qT)
                            maybe_emit_decode()
                if dec_pending[0] is not None:
                    emit_dec_av(*dec_pending[0])
                    dec_pending[0] = None
                nc.gpsimd.dma_start(ddec[:, :, :], dstage)

    nc.finalize()
    return nc


def kernel(q, k, v, k_cache, v_cache, slot_mapping, context_lens,
           decode_block_tables, **_unused):
    q = np.asarray(q, dtype=np.float32)
    k = np.asarray(k, dtype=np.float32)
    v = np.asarray(v, dtype=np.float32)
    k_cache = np.asarray(k_cache, dtype=np.float32)
    v_cache = np.asarray(v_cache, dtype=np.float32)
    slot_mapping = np.asarray(slot_mapping)
    context_lens = np.asarray(context_lens)
    decode_block_tables = np.asarray(decode_block_tables)

    # ---- host prep: apply the kv-cache scatter (the reference's
    # _store_kvcache) so decode reads the updated cache ----
    kc = k_cache.reshape(NUM_BLOCKS * BLOCK_SIZE, NUM_KV_HEADS, HEAD_DIM).copy()
    vc = v_cache.reshape(NUM_BLOCKS * BLOCK_SIZE, NUM_KV_HEADS, HEAD_DIM).copy()
    kc[slot_mapping] = k
    vc[slot_mapping] = v
    kc = kc.reshape(NUM_BLOCKS, BLOCK_SIZE, NUM_KV_HEADS, HEAD_DIM)
    vc = vc.reshape(NUM_BLOCKS, BLOCK_SIZE, NUM_KV_HEADS, HEAD_DIM)

    qpre = q[:N_PREFILL].reshape(NUM_SEQS, SEQLEN, NUM_HEADS, HEAD_DIM)
    kpre = k[:N_PREFILL].reshape(NUM_SEQS, SEQLEN, NUM_KV_HEADS, HEAD_DIM)
    vpre = v[:N_PREFILL].reshape(NUM_SEQS, SEQLEN, NUM_KV_HEADS, HEAD_DIM)
    qdec = q[N_PREFILL:]  # [32, 32, 128]

    ones_pre = np.ones((NUM_SEQS, SEQLEN, 1), np.float32)
    trimask = (np.arange(128)[:, None] <= np.arange(128)[None, :]) \
        .astype(np.float16)
    ntiles_b = (-(-context_lens.astype(np.int64) // 128)).astype(np.int64)
    rem_b = context_lens.astype(np.int64) - 128 * (ntiles_b - 1)
    tailmask = (np.arange(128)[:, None] < rem_b[None, :]).astype(np.float32)
    nblocks_b = -(-context_lens.astype(np.int64) // BLOCK_SIZE)
    tot_tiles = int(ntiles_b.sum())

    in_maps = []
    for c in range(N_CORES):
        h0 = c * GQA
        # [s, hp, d, hh, q]
        qpreT = np.ascontiguousarray(
            qpre[:, :, h0:h0 + GQA, :]
            .reshape(NUM_SEQS, SEQLEN, 2, 2, HEAD_DIM)
            .transpose(0, 2, 4, 3, 1)).astype(np.float16)
        kpreT = np.ascontiguousarray(
            kpre[:, :, c, :].transpose(0, 2, 1)).astype(np.float16)
        vpre1 = np.ascontiguousarray(
            np.concatenate([vpre[:, :, c, :], ones_pre], axis=2)
            .reshape(NUM_SEQS, SEQLEN // 128, 128, HEAD_DIM + 1)
            .transpose(0, 2, 1, 3)).astype(np.float16)
        qdecT = np.ascontiguousarray(
            qdec[:, h0:h0 + GQA, :].transpose(2, 0, 1)
            .reshape(HEAD_DIM, DECODE_BATCH * GQA)).astype(NP_FP8)
        # decode pages packed at 128-token granularity, per seq
        kparts, vparts = [], []
        for b in range(DECODE_BATCH):
            nb = int(nblocks_b[b])
            ntok = int(ntiles_b[b]) * 128
            kg = kc[decode_block_tables[b, :nb], :, c, :] \
                .reshape(nb * BLOCK_SIZE, HEAD_DIM)[:ntok]
            vg = vc[decode_block_tables[b, :nb], :, c, :] \
                .reshape(nb * BLOCK_SIZE, HEAD_DIM)[:ntok]
            kparts.append(kg)
            vparts.append(
                np.concatenate([vg, np.ones((ntok, 1), np.float32)], axis=1))
        kdec = np.ascontiguousarray(
            np.concatenate(kparts, axis=0).T).astype(NP_FP8)
        vdec = np.ascontiguousarray(
            np.concatenate(vparts, axis=0)
            .reshape(tot_tiles, 128, HEAD_DIM + 1)
            .transpose(1, 0, 2)).astype(NP_FP8)
        in_maps.append({
            "qpreT": qpreT, "kpreT": kpreT, "vpre1": vpre1,
            "qdecT": qdecT, "kdec": kdec, "vdec": vdec, "trimask": trimask,
            "tailmask": tailmask,
        })

    key = (np.ascontiguousarray(context_lens).tobytes()
           + np.ascontiguousarray(decode_block_tables).tobytes())
    nc = _program_cache.get(key)
    if nc is None:
        nc = _build_program(context_lens)
        _program_cache[key] = nc

    res = run_bass_kernel_spmd(nc, in_maps, core_ids=list(range(N_CORES)))

    out = np.empty((TOTAL, NUM_HEADS, HEAD_DIM), np.float32)
    for c in range(N_CORES):
        # prefill: [s, hp, ch, qp, hh*4+ml, 129] -> [s, ch, ml, qp, hp, hh, d]
        po = res.results[c]["preout"].astype(np.float32).reshape(
            NUM_SEQS, 2, 2, 128, 2, 4, HEAD_DIM + 1)
        po = po.transpose(0, 2, 5, 3, 1, 4, 6).reshape(
            N_PREFILL, GQA, HEAD_DIM + 1)
        out[:N_PREFILL, c * GQA:(c + 1) * GQA, :] = \
            po[:, :, :HEAD_DIM] / po[:, :, HEAD_DIM:]
        # decode: [gqa, b, 129]
        dd = res.results[c]["ddec"]
        out[N_PREFILL:, c * GQA:(c + 1) * GQA, :] = \
            (dd[:, :, :HEAD_DIM] / dd[:, :, HEAD_DIM:]).transpose(1, 0, 2)
    return out


# revision 24
# speedup vs baseline: 1.0083x; 1.0083x over previous
"""Paged GQA attention (prefill + decode) for 8 Trainium2 NeuronCores.

Sharding: tensor-parallel over kv-heads. Core c owns kv-head c and its 4 GQA
query heads. Block tables / context lens are baked into the program (compiled
per call), so all control flow and gather addresses are static.

v2 design (vs the 219us baseline):
  - all prefill operands fp16: halves input DMA and enables FWL weight loads
    (fp32r stationaries can't use FWL, serializing 107ns LDWEIGHTS per matmul)
  - QK matmuls causally column-sliced (25% fewer PE columns)
  - exp merged across 2 query heads per instruction (each ACTIVATE carries a
    ~352-cycle fixed overhead; merging halves the count) with causal column
    skipping via 3D strided APs
  - outputs written unnormalized (with the ones-column row sums) as fp16;
    the division happens on host: frees ~90us of VectorE work and halves
    output DMA
  - decode uses fp8e4m3 KV packed at 128-token granularity, loaded in 2 large
    DMAs at program start (instead of 64 small ones), and runs as a separate
    phase at the end so prefill PSUM pools can be released and reused
"""

import sys

if "/opt/trn_rl_repo" not in sys.path:
    sys.path.insert(0, "/opt/trn_rl_repo")

import numpy as np
import ml_dtypes

import concourse.bass as bass  # noqa: F401  (registers AP machinery)
import concourse.mybir as mybir
import concourse.tile as tile
from concourse import bacc
from concourse.bass_utils import run_bass_kernel_spmd

NUM_HEADS = 32
NUM_KV_HEADS = 8
HEAD_DIM = 128
GQA = NUM_HEADS // NUM_KV_HEADS  # 4
SCALE = 0.08838834764831845
NUM_SEQS = 4
SEQLEN = 1024
N_PREFILL = NUM_SEQS * SEQLEN  # 4096
DECODE_BATCH = 32
NUM_BLOCKS = 256
BLOCK_SIZE = 256
MAX_BLOCKS = 8
TOTAL = N_PREFILL + DECODE_BATCH  # 4128
N_CORES = 8
MAX_KTILES = 16  # ceil(2047/128)

F32 = mybir.dt.float32
BF16 = mybir.dt.bfloat16
FP16 = mybir.dt.float16
FP8 = mybir.dt.float8e4
EXP = mybir.ActivationFunctionType.Exp

NP_FP8 = ml_dtypes.float8_e4m3fn

_program_cache: dict[bytes, object] = {}


def _build_program(ctx_lens: np.ndarray):
    """Build + finalize the (SPMD-identical) Bass program for one core."""
    nc = bacc.Bacc("TRN2", target_bir_lowering=False)

    # ---- static decode geometry (baked) ----
    ntiles_b = [-(-int(ctx_lens[b]) // 128) for b in range(DECODE_BATCH)]
    tile_off = [0]
    for nt in ntiles_b:
        tile_off.append(tile_off[-1] + nt)
    tot_tiles = tile_off[-1]

    qpreT = nc.dram_tensor("qpreT", [NUM_SEQS, 2, HEAD_DIM, 2, SEQLEN], FP16,
                           kind="ExternalInput")
    kpreT = nc.dram_tensor("kpreT", [NUM_SEQS, HEAD_DIM, SEQLEN], FP16,
                           kind="ExternalInput")
    vpre1 = nc.dram_tensor(
        "vpre1", [NUM_SEQS, 128, SEQLEN // 128, HEAD_DIM + 1], FP16,
        kind="ExternalInput")
    qdecT = nc.dram_tensor("qdecT", [HEAD_DIM, DECODE_BATCH * GQA], FP8,
                           kind="ExternalInput")
    kdec = nc.dram_tensor("kdec", [HEAD_DIM, tot_tiles * 128], FP8,
                          kind="ExternalInput")
    vdec = nc.dram_tensor("vdec", [128, tot_tiles, HEAD_DIM + 1], FP8,
                          kind="ExternalInput")
    trimask = nc.dram_tensor("trimask", [128, 128], FP16, kind="ExternalInput")
    tailmask = nc.dram_tensor("tailmask", [128, DECODE_BATCH], F32,
                              kind="ExternalInput")
    # unnormalized prefill out: [s, hp, c, 128 q, hh*4+ml, 129]
    preout = nc.dram_tensor(
        "preout", [NUM_SEQS, 2, 2, 128, 8, HEAD_DIM + 1], FP16,
        kind="ExternalOutput")
    # unnormalized decode out: [4 gqa, 32 seq, 129]
    ddec = nc.dram_tensor("ddec", [GQA, DECODE_BATCH, HEAD_DIM + 1], F32,
                          kind="ExternalOutput")

    with tile.TileContext(nc) as tc:
        with tc.tile_pool(name="consts", bufs=1) as consts, \
             tc.tile_pool(name="kv8", bufs=1) as kv8_pool:
            tri = consts.tile([128, 128], FP16)
            nc.sync.dma_start(tri, trimask[:, :])
            tail_s = consts.tile([128, DECODE_BATCH], F32)
            with tc.tile_wait_until(0.012):
                nc.sync.dma_start(tail_s, tailmask[:, :])
            qdec_s = consts.tile([HEAD_DIM, DECODE_BATCH * GQA], FP8)
            with tc.tile_wait_until(0.012):
                nc.sync.dma_start(qdec_s, qdecT[:, :])
            kp = kv8_pool.tile([HEAD_DIM, tot_tiles * 128], FP8, name="kp")
            vp = kv8_pool.tile([128, tot_tiles, HEAD_DIM + 1], FP8, name="vp")
            kv_prefetch_done = [False]
            # HAM warmup: ~5us of dummy back-to-back matmuls while the real
            # inputs stream in, so the PE clock gate is already at 8/8 when
            # the first real matmul issues (saves ~10us of half-clock start)
            wz = consts.tile([128, 512], FP16, name="wz")
            nc.vector.memset(wz, 0.0)

            # ---------------- prefill + interleaved decode ----------------
            # bufs cover the whole problem: all inputs prefetch at t=0 so no
            # mid-kernel load can be starved by the decode-KV stream
            with tc.tile_pool(name="kT", bufs=4) as kT_pool, \
                 tc.tile_pool(name="v1", bufs=4) as v1_pool, \
                 tc.tile_pool(name="qT", bufs=8) as qT_pool, \
                 tc.tile_pool(name="es", bufs=3) as e_pool, \
                 tc.tile_pool(name="stg", bufs=3) as stg_pool, \
                 tc.tile_pool(name="ed", bufs=2) as ed_pool, \
                 tc.tile_pool(name="dst", bufs=1) as dst_pool, \
                 tc.tile_pool(name="sc", bufs=2, space="PSUM") as s_pool, \
                 tc.tile_pool(name="ot", bufs=1, space="PSUM") as o_pool, \
                 tc.tile_pool(name="dec", bufs=1, space="PSUM") as dec_pool:
                dstage = dst_pool.tile([GQA, DECODE_BATCH, HEAD_DIM + 1], F32)

                def emit_chunk(s, hp, c, kT, v1, qT):
                    nj = 4 * (c + 1)
                    # 8 otile slots (hh*4+ml) packed 3/3/2 per PSUM bank
                    otA = o_pool.tile([128, 3, HEAD_DIM + 1], F32, name="otA",
                                      tag="otA")
                    otB = o_pool.tile([128, 3, HEAD_DIM + 1], F32, name="otB",
                                      tag="otB")
                    otC = o_pool.tile([128, 2, HEAD_DIM + 1], F32, name="otC",
                                      tag="otC")

                    def ot_slice(idx):
                        if idx < 3:
                            return otA[:, idx, :]
                        if idx < 6:
                            return otB[:, idx - 3, :]
                        return otC[:, idx - 6, :]

                    stage = stg_pool.tile([128, 8, HEAD_DIM + 1], FP16,
                                          name="stage")
                    es = []

                    def emit_av(j, e):
                        ml0 = j - 4 * c if j > 4 * c else 0
                        for hh in range(2):
                            for ml in range(ml0, 4):
                                idx = hh * 4 + ml
                                # start=True clears has_written for the WHOLE
                                # PSUM bank, so only the first group written to
                                # each bank (idx 0/3/6) may set it; the other
                                # groups' first writes land on cleared bits and
                                # overwrite anyway.
                                nc.tensor.matmul(
                                    ot_slice(idx),
                                    e[:, hh, ml * 128:(ml + 1) * 128],
                                    v1[:, j, :],
                                    start=(j == 0 and idx in (0, 3, 6)),
                                    stop=(j == 4 * c + ml),
                                    skip_group_check=True)
                                if j == 4 * c + ml:
                                    # evacuate as soon as this otile stops
                                    # (DVE: gpsimd can't read PSUM)
                                    nc.vector.tensor_copy(
                                        stage[:, idx, :], ot_slice(idx))

                    for j in range(nj):
                        off = 128 * (j - 4 * c) if j > 4 * c else 0
                        sc = s_pool.tile([128, 2, 512], F32, name="sc")
                        for hh in range(2):
                            nc.tensor.matmul(
                                sc[:, hh, off:],
                                kT[:, j * 128:(j + 1) * 128],
                                qT[:, hh, c * 512 + off:(c + 1) * 512],
                                start=True, stop=True)
                        e = e_pool.tile([128, 2, 512], FP16, name="e")
                        nc.scalar.activation(
                            e[:, :, off:], sc[:, :, off:], EXP, scale=SCALE)
                        if j >= 4 * c:
                            for hh in range(2):
                                nc.vector.tensor_mul(
                                    e[:, hh, off:off + 128],
                                    e[:, hh, off:off + 128],
                                    tri)
                        es.append((j, e))
                        if len(es) > 2:
                            emit_av(*es.pop(0))
                    while es:
                        emit_av(*es.pop(0))
                    nc.gpsimd.dma_start(preout[s, hp, c], stage)
                    if not kv_prefetch_done[0]:
                        # decode KV prefetch (9MB): delayed + split into
                        # per-4-seq slices. One huge descriptor starves the
                        # per-descriptor round-robin against the startup-
                        # critical q/k/v loads (first matmul then waits ~30us
                        # on its inputs); small descriptors share fairly and
                        # the wait hint keeps them off the critical window.
                        kv_prefetch_done[0] = True
                        for i, b0 in enumerate(range(0, DECODE_BATCH, 4)):
                            ta = tile_off[b0]
                            tb = tile_off[min(b0 + 4, DECODE_BATCH)]
                            with tc.tile_wait_until(0.030 + 0.006 * i):
                                nc.gpsimd.dma_start(
                                    kp[:, ta * 128:tb * 128],
                                    kdec[:, ta * 128:tb * 128])
                                nc.gpsimd.dma_start(
                                    vp[:, ta:tb, :], vdec[:, ta:tb, :])

                scw = dec_pool.tile([128, 512], F32, name="dec")
                for _ in range(12):
                    nc.tensor.matmul(scw, wz[:, 0:128], wz,
                                     start=True, stop=True,
                                     skip_group_check=True)

                NG = 4  # decode seqs per group
                SDW = NG * 4 * MAX_KTILES  # 256 score cols per group

                def emit_dec_qk(g):
                    # sd (cols 0:256) + od (cols 256:385) share one PSUM bank
                    # via flat offsets (PSUM slots pad to whole banks, so two
                    # tags won't pack)
                    dec = dec_pool.tile([128, 512], F32, name="dec")
                    nc.vector.memset(dec[:, 0:SDW], 0.0)
                    for i in range(NG):
                        b = g * NG + i
                        for t in range(ntiles_b[b]):
                            gt = tile_off[b] + t
                            o = i * 4 * MAX_KTILES + 4 * t
                            nc.tensor.matmul(
                                dec[:, o:o + 4],
                                kp[:, gt * 128:(gt + 1) * 128],
                                qdec_s[:, 4 * b:4 * b + 4],
                                start=True, stop=True,
                                skip_group_check=True)
                    ed = ed_pool.tile([128, SDW], FP16, name="ed")
                    nc.scalar.activation(ed, dec[:, 0:SDW], EXP, scale=SCALE)
                    return dec, ed

                def emit_dec_av(g, dec, ed):
                    od = dec[:, SDW:SDW + HEAD_DIM + 1]
                    for i in range(NG):
                        b = g * NG + i
                        nt = ntiles_b[b]
                        rem = int(ctx_lens[b]) - 128 * (nt - 1)
                        o = i * 4 * MAX_KTILES
                        if rem < 128:
                            nc.vector.tensor_scalar_mul(
                                ed[:, o + 4 * (nt - 1):o + 4 * nt],
                                ed[:, o + 4 * (nt - 1):o + 4 * nt],
                                tail_s[:, b:b + 1])
                        for t in range(nt):
                            gt = tile_off[b] + t
                            nc.tensor.matmul(
                                od[0:GQA, :],
                                ed[:, o + 4 * t:o + 4 * t + 4],
                                vp[:, gt, :],
                                start=(t == 0), stop=(t == nt - 1),
                                skip_group_check=True)
                        nc.vector.tensor_copy(dstage[:, b, :], od[0:GQA, :])

                # decode groups ride the back half of the prefill chunk list:
                # a full prefill chunk's matmuls sit between each group's QK
                # and its AV, hiding the exp latency, and the decode work
                # fills the PE slack of the ScalarE-paced prefill chunks.
                n_groups = DECODE_BATCH // NG
                chunk_no = [0]
                dec_pending = [None]

                def maybe_emit_decode():
                    k = chunk_no[0]
                    chunk_no[0] += 1
                    g = k - 5  # groups ride chunks 5..12: clear of both the
                    # startup-critical loads and the final-chunk tail
                    if dec_pending[0] is not None:
                        emit_dec_av(*dec_pending[0])
                        dec_pending[0] = None
                    if 0 <= g < n_groups:
                        dec, ed = emit_dec_qk(g)
                        dec_pending[0] = (g, dec, ed)

                for s in range(NUM_SEQS):
                    kT = kT_pool.tile([128, SEQLEN], FP16, name="kT")
                    # halves: the first chunk only needs k-tiles 0..3
                    nc.sync.dma_start(kT[:, 0:512], kpreT[s][:, 0:512])
                    nc.sync.dma_start(kT[:, 512:], kpreT[s][:, 512:])
                    for hp in range(2):
                        qT = qT_pool.tile([128, 2, SEQLEN], FP16, name="qT")
                        # split per (head, chunk) so the first matmul only
                        # waits on a 128KB load
                        for hh in range(2):
                            nc.sync.dma_start(
                                qT[:, hh, 0:512], qpreT[s, hp, :, hh, 0:512])
                            nc.sync.dma_start(
                                qT[:, hh, 512:], qpreT[s, hp, :, hh, 512:])
                        if hp == 0:
                            v1 = v1_pool.tile(
                                [128, SEQLEN // 128, HEAD_DIM + 1], FP16,
                                name="v1")
                            nc.sync.dma_start(v1, vpre1[s])
                        for c in range(2):
                            emit_chunk(s, hp, c, kT, v1, qT)
                            maybe_emit_decode()
                if dec_pending[0] is not None:
                    emit_dec_av(*dec_pending[0])
                    dec_pending[0] = None
                nc.gpsimd.dma_start(ddec[:, :, :], dstage)

    nc.finalize()
    return nc


def kernel(q, k, v, k_cache, v_cache, slot_mapping, context_lens,
           decode_block_tables, **_unused):
    q = np.asarray(q, dtype=np.float32)
    k = np.asarray(k, dtype=np.float32)
    v = np.asarray(v, dtype=np.float32)
    k_cache = np.asarray(k_cache, dtype=np.float32)
    v_cache = np.asarray(v_cache, dtype=np.float32)
    slot_mapping = np.asarray(slot_mapping)
    context_lens = np.asarray(context_lens)
    decode_block_tables = np.asarray(decode_block_tables)

    # ---- host prep: apply the kv-cache scatter (the reference's
    # _store_kvcache) so decode reads the updated cache ----
    kc = k_cache.reshape(NUM_BLOCKS * BLOCK_SIZE, NUM_KV_HEADS, HEAD_DIM).copy()
    vc = v_cache.reshape(NUM_BLOCKS * BLOCK_SIZE, NUM_KV_HEADS, HEAD_DIM).copy()
    kc[slot_mapping] = k
    vc[slot_mapping] = v
    kc = kc.reshape(NUM_BLOCKS, BLOCK_SIZE, NUM_KV_HEADS, HEAD_DIM)
    vc = vc.reshape(NUM_BLOCKS, BLOCK_SIZE, NUM_KV_HEADS, HEAD_DIM)

    qpre = q[:N_PREFILL].reshape(NUM_SEQS, SEQLEN, NUM_HEADS, HEAD_DIM)
    kpre = k[:N_PREFILL].reshape(NUM_SEQS, SEQLEN, NUM_KV_HEADS, HEAD_DIM)
    vpre = v[:N_PREFILL].reshape(NUM_SEQS, SEQLEN, NUM_KV_HEADS, HEAD_DIM)
    qdec = q[N_PREFILL:]  # [32, 32, 128]

    ones_pre = np.ones((NUM_SEQS, SEQLEN, 1), np.float32)
    trimask = (np.arange(128)[:, None] <= np.arange(128)[None, :]) \
        .astype(np.float16)
    ntiles_b = (-(-context_lens.astype(np.int64) // 128)).astype(np.int64)
    rem_b = context_lens.astype(np.int64) - 128 * (ntiles_b - 1)
    tailmask = (np.arange(128)[:, None] < rem_b[None, :]).astype(np.float32)
    nblocks_b = -(-context_lens.astype(np.int64) // BLOCK_SIZE)
    tot_tiles = int(ntiles_b.sum())

    in_maps = []
    for c in range(N_CORES):
        h0 = c * GQA
        # [s, hp, d, hh, q]
        qpreT = np.ascontiguousarray(
            qpre[:, :, h0:h0 + GQA, :]
            .reshape(NUM_SEQS, SEQLEN, 2, 2, HEAD_DIM)
            .transpose(0, 2, 4, 3, 1)).astype(np.float16)
        kpreT = np.ascontiguousarray(
            kpre[:, :, c, :].transpose(0, 2, 1)).astype(np.float16)
        vpre1 = np.ascontiguousarray(
            np.concatenate([vpre[:, :, c, :], ones_pre], axis=2)
            .reshape(NUM_SEQS, SEQLEN // 128, 128, HEAD_DIM + 1)
            .transpose(0, 2, 1, 3)).astype(np.float16)
        qdecT = np.ascontiguousarray(
            qdec[:, h0:h0 + GQA, :].transpose(2, 0, 1)
            .reshape(HEAD_DIM, DECODE_BATCH * GQA)).astype(NP_FP8)
        # decode pages packed at 128-token granularity, per seq
        kparts, vparts = [], []
        for b in range(DECODE_BATCH):
            nb = int(nblocks_b[b])
            ntok = int(ntiles_b[b]) * 128
            kg = kc[decode_block_tables[b, :nb], :, c, :] \
                .reshape(nb * BLOCK_SIZE, HEAD_DIM)[:ntok]
            vg = vc[decode_block_tables[b, :nb], :, c, :] \
                .reshape(nb * BLOCK_SIZE, HEAD_DIM)[:ntok]
            kparts.append(kg)
            vparts.append(
                np.concatenate([vg, np.ones((ntok, 1), np.float32)], axis=1))
        kdec = np.ascontiguousarray(
            np.concatenate(kparts, axis=0).T).astype(NP_FP8)
        vdec = np.ascontiguousarray(
            np.concatenate(vparts, axis=0)
            .reshape(tot_tiles, 128, HEAD_DIM + 1)
            .transpose(1, 0, 2)).astype(NP_FP8)
        in_maps.append({
            "qpreT": qpreT, "kpreT": kpreT, "vpre1": vpre1,
            "qdecT": qdecT, "kdec": kdec, "vdec": vdec, "trimask": trimask,
            "tailmask": tailmask,
        })

    key = (np.ascontiguousarray(context_lens).tobytes()
           + np.ascontiguousarray(decode_block_tables).tobytes())
    nc = _program_cache.get(key)
    if nc is None:
        nc = _build_program(context_lens)
        _program_cache[key] = nc

    res = run_bass_kernel_spmd(nc, in_maps, core_ids=list(range(N_CORES)))

    out = np.empty((TOTAL, NUM_HEADS, HEAD_DIM), np.float32)
    for c in range(N_CORES):
        # prefill: [s, hp, ch, qp, hh*4+ml, 129] -> [s, ch, ml, qp, hp, hh, d]
        po = res.results[c]["preout"].astype(np.float32).reshape(
            NUM_SEQS, 2, 2, 128, 2, 4, HEAD_DIM + 1)
        po = po.transpose(0, 2, 5, 3, 1, 4, 6).reshape(
            N_PREFILL, GQA, HEAD_DIM + 1)
        out[:N_PREFILL, c * GQA:(c + 1) * GQA, :] = \
            po[:, :, :HEAD_DIM] / po[:, :, HEAD_DIM:]
        # decode: [gqa, b, 129]
        dd = res.results[c]["ddec"]
        out[N_PREFILL:, c * GQA:(c + 1) * GQA, :] = \
            (dd[:, :, :HEAD_DIM] / dd[:, :, HEAD_DIM:]).transpose(1, 0, 2)
    return out


# revision 26
# speedup vs baseline: 1.0197x; 1.0114x over previous
"""Paged GQA attention (prefill + decode) for 8 Trainium2 NeuronCores.

Sharding: tensor-parallel over kv-heads. Core c owns kv-head c and its 4 GQA
query heads. Block tables / context lens are baked into the program (compiled
per call), so all control flow and gather addresses are static.

v2 design (vs the 219us baseline):
  - all prefill operands fp16: halves input DMA and enables FWL weight loads
    (fp32r stationaries can't use FWL, serializing 107ns LDWEIGHTS per matmul)
  - QK matmuls causally column-sliced (25% fewer PE columns)
  - exp merged across 2 query heads per instruction (each ACTIVATE carries a
    ~352-cycle fixed overhead; merging halves the count) with causal column
    skipping via 3D strided APs
  - outputs written unnormalized (with the ones-column row sums) as fp16;
    the division happens on host: frees ~90us of VectorE work and halves
    output DMA
  - decode uses fp8e4m3 KV packed at 128-token granularity, loaded in 2 large
    DMAs at program start (instead of 64 small ones), and runs as a separate
    phase at the end so prefill PSUM pools can be released and reused
"""

import sys

if "/opt/trn_rl_repo" not in sys.path:
    sys.path.insert(0, "/opt/trn_rl_repo")

import numpy as np
import ml_dtypes

import concourse.bass as bass  # noqa: F401  (registers AP machinery)
import concourse.mybir as mybir
import concourse.tile as tile
from concourse import bacc
from concourse.bass_utils import run_bass_kernel_spmd

NUM_HEADS = 32
NUM_KV_HEADS = 8
HEAD_DIM = 128
GQA = NUM_HEADS // NUM_KV_HEADS  # 4
SCALE = 0.08838834764831845
NUM_SEQS = 4
SEQLEN = 1024
N_PREFILL = NUM_SEQS * SEQLEN  # 4096
DECODE_BATCH = 32
NUM_BLOCKS = 256
BLOCK_SIZE = 256
MAX_BLOCKS = 8
TOTAL = N_PREFILL + DECODE_BATCH  # 4128
N_CORES = 8
MAX_KTILES = 16  # ceil(2047/128)

F32 = mybir.dt.float32
BF16 = mybir.dt.bfloat16
FP16 = mybir.dt.float16
FP8 = mybir.dt.float8e4
EXP = mybir.ActivationFunctionType.Exp

NP_FP8 = ml_dtypes.float8_e4m3fn

_program_cache: dict[bytes, object] = {}


def _build_program(ctx_lens: np.ndarray):
    """Build + finalize the (SPMD-identical) Bass program for one core."""
    nc = bacc.Bacc("TRN2", target_bir_lowering=False)

    # ---- static decode geometry (baked) ----
    ntiles_b = [-(-int(ctx_lens[b]) // 128) for b in range(DECODE_BATCH)]
    tile_off = [0]
    for nt in ntiles_b:
        tile_off.append(tile_off[-1] + nt)
    tot_tiles = tile_off[-1]

    qpreT = nc.dram_tensor("qpreT", [NUM_SEQS, 2, HEAD_DIM, 2, SEQLEN], FP16,
                           kind="ExternalInput")
    kpreT = nc.dram_tensor("kpreT", [NUM_SEQS, HEAD_DIM, SEQLEN], FP16,
                           kind="ExternalInput")
    vpre1 = nc.dram_tensor(
        "vpre1", [NUM_SEQS, 128, SEQLEN // 128, HEAD_DIM + 1], FP16,
        kind="ExternalInput")
    qdecT = nc.dram_tensor("qdecT", [HEAD_DIM, DECODE_BATCH * GQA], FP8,
                           kind="ExternalInput")
    kdec = nc.dram_tensor("kdec", [HEAD_DIM, tot_tiles * 128], FP8,
                          kind="ExternalInput")
    vdec = nc.dram_tensor("vdec", [128, tot_tiles, HEAD_DIM + 1], FP8,
                          kind="ExternalInput")
    trimask = nc.dram_tensor("trimask", [128, 128], FP16, kind="ExternalInput")
    tailmask = nc.dram_tensor("tailmask", [128, DECODE_BATCH], F32,
                              kind="ExternalInput")
    # unnormalized prefill out: [s, hp, c, 128 q, hh*4+ml, 129]
    preout = nc.dram_tensor(
        "preout", [NUM_SEQS, 2, 2, 128, 8, HEAD_DIM + 1], FP16,
        kind="ExternalOutput")
    # unnormalized decode out: [4 gqa, 32 seq, 129]
    ddec = nc.dram_tensor("ddec", [GQA, DECODE_BATCH, HEAD_DIM + 1], F32,
                          kind="ExternalOutput")

    with tile.TileContext(nc) as tc:
        with tc.tile_pool(name="consts", bufs=1) as consts, \
             tc.tile_pool(name="kv8", bufs=1) as kv8_pool:
            tri = consts.tile([128, 128], FP16)
            nc.sync.dma_start(tri, trimask[:, :])
            tail_s = consts.tile([128, DECODE_BATCH], F32)
            with tc.tile_wait_until(0.012):
                nc.sync.dma_start(tail_s, tailmask[:, :])
            qdec_s = consts.tile([HEAD_DIM, DECODE_BATCH * GQA], FP8)
            with tc.tile_wait_until(0.012):
                nc.sync.dma_start(qdec_s, qdecT[:, :])
            kp = kv8_pool.tile([HEAD_DIM, tot_tiles * 128], FP8, name="kp")
            vp = kv8_pool.tile([128, tot_tiles, HEAD_DIM + 1], FP8, name="vp")
            kv_prefetch_done = [False]
            # HAM warmup: ~5us of dummy back-to-back matmuls while the real
            # inputs stream in, so the PE clock gate is already at 8/8 when
            # the first real matmul issues (saves ~10us of half-clock start)
            wz = consts.tile([128, 512], FP16, name="wz")
            nc.vector.memset(wz, 0.0)

            # ---------------- prefill + interleaved decode ----------------
            # bufs cover the whole problem: all inputs prefetch at t=0 so no
            # mid-kernel load can be starved by the decode-KV stream
            with tc.tile_pool(name="kT", bufs=4) as kT_pool, \
                 tc.tile_pool(name="v1", bufs=4) as v1_pool, \
                 tc.tile_pool(name="qT", bufs=8) as qT_pool, \
                 tc.tile_pool(name="es", bufs=4) as e_pool, \
                 tc.tile_pool(name="stg", bufs=4) as stg_pool, \
                 tc.tile_pool(name="ed", bufs=2) as ed_pool, \
                 tc.tile_pool(name="dst", bufs=1) as dst_pool, \
                 tc.tile_pool(name="sc", bufs=2, space="PSUM") as s_pool, \
                 tc.tile_pool(name="ot", bufs=1, space="PSUM") as o_pool, \
                 tc.tile_pool(name="dec", bufs=1, space="PSUM") as dec_pool:
                dstage = dst_pool.tile([GQA, DECODE_BATCH, HEAD_DIM + 1], F32)

                def emit_chunk(s, hp, c, kT, v1, qT):
                    nj = 4 * (c + 1)
                    # 8 otile slots (hh*4+ml) packed 3/3/2 per PSUM bank
                    otA = o_pool.tile([128, 3, HEAD_DIM + 1], F32, name="otA",
                                      tag="otA")
                    otB = o_pool.tile([128, 3, HEAD_DIM + 1], F32, name="otB",
                                      tag="otB")
                    otC = o_pool.tile([128, 2, HEAD_DIM + 1], F32, name="otC",
                                      tag="otC")

                    def ot_slice(idx):
                        if idx < 3:
                            return otA[:, idx, :]
                        if idx < 6:
                            return otB[:, idx - 3, :]
                        return otC[:, idx - 6, :]

                    stage = stg_pool.tile([128, 8, HEAD_DIM + 1], FP16,
                                          name="stage")
                    es = []

                    def emit_av(j, e):
                        ml0 = j - 4 * c if j > 4 * c else 0
                        for hh in range(2):
                            for ml in range(ml0, 4):
                                idx = hh * 4 + ml
                                # start=True clears has_written for the WHOLE
                                # PSUM bank, so only the first group written to
                                # each bank (idx 0/3/6) may set it; the other
                                # groups' first writes land on cleared bits and
                                # overwrite anyway.
                                nc.tensor.matmul(
                                    ot_slice(idx),
                                    e[:, hh, ml * 128:(ml + 1) * 128],
                                    v1[:, j, :],
                                    start=(j == 0 and idx in (0, 3, 6)),
                                    stop=(j == 4 * c + ml),
                                    skip_group_check=True)
                                if j == 4 * c + ml:
                                    # evacuate as soon as this otile stops
                                    # (DVE: gpsimd can't read PSUM)
                                    nc.vector.tensor_copy(
                                        stage[:, idx, :], ot_slice(idx))

                    for j in range(nj):
                        off = 128 * (j - 4 * c) if j > 4 * c else 0
                        sc = s_pool.tile([128, 2, 512], F32, name="sc")
                        for hh in range(2):
                            nc.tensor.matmul(
                                sc[:, hh, off:],
                                kT[:, j * 128:(j + 1) * 128],
                                qT[:, hh, c * 512 + off:(c + 1) * 512],
                                start=True, stop=True)
                        e = e_pool.tile([128, 2, 512], FP16, name="e")
                        nc.scalar.activation(
                            e[:, :, off:], sc[:, :, off:], EXP, scale=SCALE)
                        if j >= 4 * c:
                            for hh in range(2):
                                nc.vector.tensor_mul(
                                    e[:, hh, off:off + 128],
                                    e[:, hh, off:off + 128],
                                    tri)
                        es.append((j, e))
                        if len(es) > 2:
                            emit_av(*es.pop(0))
                    while es:
                        emit_av(*es.pop(0))
                    nc.gpsimd.dma_start(preout[s, hp, c], stage)
                    if not kv_prefetch_done[0]:
                        # decode KV prefetch (9MB): delayed + split into
                        # per-4-seq slices. One huge descriptor starves the
                        # per-descriptor round-robin against the startup-
                        # critical q/k/v loads (first matmul then waits ~30us
                        # on its inputs); small descriptors share fairly and
                        # the wait hint keeps them off the critical window.
                        kv_prefetch_done[0] = True
                        for i, b0 in enumerate(range(0, DECODE_BATCH, 4)):
                            ta = tile_off[b0]
                            tb = tile_off[min(b0 + 4, DECODE_BATCH)]
                            with tc.tile_wait_until(0.030 + 0.006 * i):
                                nc.gpsimd.dma_start(
                                    kp[:, ta * 128:tb * 128],
                                    kdec[:, ta * 128:tb * 128])
                                nc.gpsimd.dma_start(
                                    vp[:, ta:tb, :], vdec[:, ta:tb, :])

                scw = dec_pool.tile([128, 512], F32, name="dec")
                for _ in range(7):
                    nc.tensor.matmul(scw, wz[:, 0:128], wz,
                                     start=True, stop=True,
                                     skip_group_check=True)

                NG = 4  # decode seqs per group
                SDW = NG * 4 * MAX_KTILES  # 256 score cols per group

                def emit_dec_qk(g):
                    # sd (cols 0:256) + od (cols 256:385) share one PSUM bank
                    # via flat offsets (PSUM slots pad to whole banks, so two
                    # tags won't pack)
                    dec = dec_pool.tile([128, 512], F32, name="dec")
                    nc.vector.memset(dec[:, 0:SDW], 0.0)
                    for i in range(NG):
                        b = g * NG + i
                        for t in range(ntiles_b[b]):
                            gt = tile_off[b] + t
                            o = i * 4 * MAX_KTILES + 4 * t
                            nc.tensor.matmul(
                                dec[:, o:o + 4],
                                kp[:, gt * 128:(gt + 1) * 128],
                                qdec_s[:, 4 * b:4 * b + 4],
                                start=True, stop=True,
                                skip_group_check=True)
                    ed = ed_pool.tile([128, SDW], FP16, name="ed")
                    nc.scalar.activation(ed, dec[:, 0:SDW], EXP, scale=SCALE)
                    return dec, ed

                def emit_dec_av(g, dec, ed):
                    od = dec[:, SDW:SDW + HEAD_DIM + 1]
                    for i in range(NG):
                        b = g * NG + i
                        nt = ntiles_b[b]
                        rem = int(ctx_lens[b]) - 128 * (nt - 1)
                        o = i * 4 * MAX_KTILES
                        if rem < 128:
                            nc.vector.tensor_scalar_mul(
                                ed[:, o + 4 * (nt - 1):o + 4 * nt],
                                ed[:, o + 4 * (nt - 1):o + 4 * nt],
                                tail_s[:, b:b + 1])
                        for t in range(nt):
                            gt = tile_off[b] + t
                            nc.tensor.matmul(
                                od[0:GQA, :],
                                ed[:, o + 4 * t:o + 4 * t + 4],
                                vp[:, gt, :],
                                start=(t == 0), stop=(t == nt - 1),
                                skip_group_check=True)
                        nc.vector.tensor_copy(dstage[:, b, :], od[0:GQA, :])

                # decode groups ride the back half of the prefill chunk list:
                # a full prefill chunk's matmuls sit between each group's QK
                # and its AV, hiding the exp latency, and the decode work
                # fills the PE slack of the ScalarE-paced prefill chunks.
                n_groups = DECODE_BATCH // NG
                chunk_no = [0]
                dec_pending = [None]

                def maybe_emit_decode():
                    k = chunk_no[0]
                    chunk_no[0] += 1
                    g = k - 5  # groups ride chunks 5..12: clear of both the
                    # startup-critical loads and the final-chunk tail
                    if dec_pending[0] is not None:
                        emit_dec_av(*dec_pending[0])
                        dec_pending[0] = None
                    if 0 <= g < n_groups:
                        dec, ed = emit_dec_qk(g)
                        dec_pending[0] = (g, dec, ed)

                for s in range(NUM_SEQS):
                    kT = kT_pool.tile([128, SEQLEN], FP16, name="kT")
                    # halves: the first chunk only needs k-tiles 0..3
                    nc.sync.dma_start(kT[:, 0:512], kpreT[s][:, 0:512])
                    nc.sync.dma_start(kT[:, 512:], kpreT[s][:, 512:])
                    for hp in range(2):
                        qT = qT_pool.tile([128, 2, SEQLEN], FP16, name="qT")
                        # split per (head, chunk) so the first matmul only
                        # waits on a 128KB load
                        for hh in range(2):
                            nc.sync.dma_start(
                                qT[:, hh, 0:512], qpreT[s, hp, :, hh, 0:512])
                            nc.sync.dma_start(
                                qT[:, hh, 512:], qpreT[s, hp, :, hh, 512:])
                        if hp == 0:
                            v1 = v1_pool.tile(
                                [128, SEQLEN // 128, HEAD_DIM + 1], FP16,
                                name="v1")
                            nc.sync.dma_start(v1, vpre1[s])
                        for c in range(2):
                            emit_chunk(s, hp, c, kT, v1, qT)
                            maybe_emit_decode()
                if dec_pending[0] is not None:
                    emit_dec_av(*dec_pending[0])
                    dec_pending[0] = None
                nc.gpsimd.dma_start(ddec[:, :, :], dstage)

    nc.finalize()
    return nc


def kernel(q, k, v, k_cache, v_cache, slot_mapping, context_lens,
           decode_block_tables, **_unused):
    q = np.asarray(q, dtype=np.float32)
    k = np.asarray(k, dtype=np.float32)
    v = np.asarray(v, dtype=np.float32)
    k_cache = np.asarray(k_cache, dtype=np.float32)
    v_cache = np.asarray(v_cache, dtype=np.float32)
    slot_mapping = np.asarray(slot_mapping)
    context_lens = np.asarray(context_lens)
    decode_block_tables = np.asarray(decode_block_tables)

    # ---- host prep: apply the kv-cache scatter (the reference's
    # _store_kvcache) so decode reads the updated cache ----
    kc = k_cache.reshape(NUM_BLOCKS * BLOCK_SIZE, NUM_KV_HEADS, HEAD_DIM).copy()
    vc = v_cache.reshape(NUM_BLOCKS * BLOCK_SIZE, NUM_KV_HEADS, HEAD_DIM).copy()
    kc[slot_mapping] = k
    vc[slot_mapping] = v
    kc = kc.reshape(NUM_BLOCKS, BLOCK_SIZE, NUM_KV_HEADS, HEAD_DIM)
    vc = vc.reshape(NUM_BLOCKS, BLOCK_SIZE, NUM_KV_HEADS, HEAD_DIM)

    qpre = q[:N_PREFILL].reshape(NUM_SEQS, SEQLEN, NUM_HEADS, HEAD_DIM)
    kpre = k[:N_PREFILL].reshape(NUM_SEQS, SEQLEN, NUM_KV_HEADS, HEAD_DIM)
    vpre = v[:N_PREFILL].reshape(NUM_SEQS, SEQLEN, NUM_KV_HEADS, HEAD_DIM)
    qdec = q[N_PREFILL:]  # [32, 32, 128]

    ones_pre = np.ones((NUM_SEQS, SEQLEN, 1), np.float32)
    trimask = (np.arange(128)[:, None] <= np.arange(128)[None, :]) \
        .astype(np.float16)
    ntiles_b = (-(-context_lens.astype(np.int64) // 128)).astype(np.int64)
    rem_b = context_lens.astype(np.int64) - 128 * (ntiles_b - 1)
    tailmask = (np.arange(128)[:, None] < rem_b[None, :]).astype(np.float32)
    nblocks_b = -(-context_lens.astype(np.int64) // BLOCK_SIZE)
    tot_tiles = int(ntiles_b.sum())

    in_maps = []
    for c in range(N_CORES):
        h0 = c * GQA
        # [s, hp, d, hh, q]
        qpreT = np.ascontiguousarray(
            qpre[:, :, h0:h0 + GQA, :]
            .reshape(NUM_SEQS, SEQLEN, 2, 2, HEAD_DIM)
            .transpose(0, 2, 4, 3, 1)).astype(np.float16)
        kpreT = np.ascontiguousarray(
            kpre[:, :, c, :].transpose(0, 2, 1)).astype(np.float16)
        vpre1 = np.ascontiguousarray(
            np.concatenate([vpre[:, :, c, :], ones_pre], axis=2)
            .reshape(NUM_SEQS, SEQLEN // 128, 128, HEAD_DIM + 1)
            .transpose(0, 2, 1, 3)).astype(np.float16)
        qdecT = np.ascontiguousarray(
            qdec[:, h0:h0 + GQA, :].transpose(2, 0, 1)
            .reshape(HEAD_DIM, DECODE_BATCH * GQA)).astype(NP_FP8)
        # decode pages packed at 128-token granularity, per seq
        kparts, vparts = [], []
        for b in range(DECODE_BATCH):
            nb = int(nblocks_b[b])
            ntok = int(ntiles_b[b]) * 128
            kg = kc[decode_block_tables[b, :nb], :, c, :] \
                .reshape(nb * BLOCK_SIZE, HEAD_DIM)[:ntok]
            vg = vc[decode_block_tables[b, :nb], :, c, :] \
                .reshape(nb * BLOCK_SIZE, HEAD_DIM)[:ntok]
            kparts.append(kg)
            vparts.append(
                np.concatenate([vg, np.ones((ntok, 1), np.float32)], axis=1))
        kdec = np.ascontiguousarray(
            np.concatenate(kparts, axis=0).T).astype(NP_FP8)
        vdec = np.ascontiguousarray(
            np.concatenate(vparts, axis=0)
            .reshape(tot_tiles, 128, HEAD_DIM + 1)
            .transpose(1, 0, 2)).astype(NP_FP8)
        in_maps.append({
            "qpreT": qpreT, "kpreT": kpreT, "vpre1": vpre1,
            "qdecT": qdecT, "kdec": kdec, "vdec": vdec, "trimask": trimask,
            "tailmask": tailmask,
        })

    key = (np.ascontiguousarray(context_lens).tobytes()
           + np.ascontiguousarray(decode_block_tables).tobytes())
    nc = _program_cache.get(key)
    if nc is None:
        nc = _build_program(context_lens)
        _program_cache[key] = nc

    res = run_bass_kernel_spmd(nc, in_maps, core_ids=list(range(N_CORES)))

    out = np.empty((TOTAL, NUM_HEADS, HEAD_DIM), np.float32)
    for c in range(N_CORES):
        # prefill: [s, hp, ch, qp, hh*4+ml, 129] -> [s, ch, ml, qp, hp, hh, d]
        po = res.results[c]["preout"].astype(np.float32).reshape(
            NUM_SEQS, 2, 2, 128, 2, 4, HEAD_DIM + 1)
        po = po.transpose(0, 2, 5, 3, 1, 4, 6).reshape(
            N_PREFILL, GQA, HEAD_DIM + 1)
        out[:N_PREFILL, c * GQA:(c + 1) * GQA, :] = \
            po[:, :, :HEAD_DIM] / po[:, :, HEAD_DIM:]
        # decode: [gqa, b, 129]
        dd = res.results[c]["ddec"]
        out[N_PREFILL:, c * GQA:(c + 1) * GQA, :] = \
            (dd[:, :, :HEAD_DIM] / dd[:, :, HEAD_DIM:]).transpose(1, 0, 2)
    return out
